# revision 56
# baseline (speedup 1.0000x reference)
"""MultiHeadAttention Trainium2 kernel (8-core SPMD).

Problem: B=2, T=2048, C=1024, H=16 heads, D=64.
  out = softmax((q Wq^T + bq)(k Wk^T + bk)^T / sqrt(D)) (v Wv^T + bv) Wo^T + bo

Sharding: core c -> (batch b = c // 4, head-group g = c % 4).  Each core
computes 4 heads (a 256-wide slice of the projection space) of one batch
element, including its partial contribution to the row-sharded output
projection.  The host sums the 4 partial outputs per batch and adds bo.

Design (measured ~292 us vs the 507 us v1 baseline):
  - Activations are pre-transposed AND pre-cast to bf16 on the HOST:
    xqt/xkt/xvt = x[b].T as [C, T] bf16.  No PE transposes, no PSUM->SBUF
    staging copies, and half the input DMA bytes.
  - Q/K projections: QT/KT[co, t] = W^T.T @ xT directly (bf16 matmul,
    fp32 psum), stored f32r with bias folded via DVE tensor_scalar_add.
  - V projected in NATURAL orientation (tokens on partitions):
    Vnat[t, ds] = xvT_tile.T @ Wv^T — feeds AV without any transpose.
    VN layout per key-tile: [4 heads x (64 V cols + 64 ones cols)]; the
    ones columns make each AV matmul also emit the softmax denominator.
  - S^T[k, q] per head pair: two row-packed matmuls (partitions 0:64 /
    64:128 -> concurrent PE row-groups) into one [128, 1024] psum pair;
    ONE exp activation covers both heads (bf16 out, scale=1/8 folded).
  - Output projection accumulates W_o slices over the 256 head dims,
    written bf16; bo is added on the host during the combine.
"""

import numpy as np

B, T, C, H, D = 2, 2048, 1024, 16, 64
NCORES = 8
GROUPS = 4              # head-groups == cores per batch element
HG = H // GROUPS        # heads per core
DS = HG * D             # per-core projection slice width (256)
TCH = 512               # token chunk (psum bank = 512 fp32)
NTCH = T // TCH         # 4
NCC = C // 128          # 8 contraction chunks
NKT = T // 128          # 16 key tiles
SCALE = float(D) ** -0.5

_NC_CACHE = None


def _emit(ctx, tc, io):
    from concourse import mybir

    nc = tc.nc
    f32 = mybir.dt.float32
    f32r = mybir.dt.float32r
    bf16 = mybir.dt.bfloat16
    EXP = mybir.ActivationFunctionType.Exp

    persist = ctx.enter_context(tc.tile_pool(name="persist", bufs=1))

    def ptile(tag, shape, dt=f32):
        return persist.tile(shape, dt, tag=tag, name=tag)

    # --- persistent SBUF tensors ---------------------------------------
    # wqkv [C, 3*DS] = wq|wk|wv packed -> ONE weight DMA; biasp [128, 260]
    # = bqs|bks|bvb packed -> one more.  Both ride the scalar HWDGE ring
    # (ACT is idle until the first exp anyway) so the sync ring starts on
    # xkt immediately.
    wqkv = ptile("wqkv", [128, NCC * 3 * DS], bf16)
    nc.scalar.dma_start(
        wqkv[:].rearrange("p (a s) -> p a s", a=NCC),
        io["wqkv"].rearrange("(a p) s -> p a s", p=128))
    wsb = {}
    for i, name in enumerate(("wq", "wk", "wv")):
        wsb[name] = [
            wqkv[:, (c * 3 + i) * DS:(c * 3 + i + 1) * DS]
            for c in range(NCC)
        ]
    # wot DMAs are emitted late (just before attention) so they queue
    # behind the x loads instead of delaying them
    wot = [ptile(f"wot{dc}", [128, C], bf16) for dc in range(2)]

    biasp = ptile("biasp", [128, 4 + DS])
    nc.scalar.dma_start(biasp[:], io["biasp"][:, :])
    bias = {"bqs": biasp[:, 0:2], "bks": biasp[:, 2:4]}
    bvb = biasp[:, 4:4 + DS]

    QT = [ptile(f"qt{i}", [128, T], f32r) for i in range(2)]
    KT = [ptile(f"kt{i}", [128, T], f32r) for i in range(2)]
    # V natural, all 16 key tiles in one buffer; per key tile the layout is
    # [4 heads x (64 V cols + 64 ones cols)].  Ones prefilled via memset;
    # V columns overwritten when the V projection lands.
    VNB = ptile("vnb", [128, NKT * 512], bf16)
    nc.gpsimd.memset(VNB[:], 1.0)

    # --- stage A + B, software-pipelined -------------------------------
    # DMA order: weights (scalar), xkt (sync), xqt (scalar), xvt (sync):
    # scores for q-chunk 0 only need K fully projected + Q chunk 0, so the
    # exp stream starts as soon as K + Q0 land; V arrives third and feeds
    # the AV matmuls (the 6-deep es pool bridges the exp->AV lag).
    # PSUM: projections 4 banks (pool closed before attention), then
    # scores 2x[128,1024] (4 banks) + 4 AV-accumulator banks (the output
    # projection borrows freed accumulator slots).
    with tc.tile_pool(name="xin", bufs=1) as xinp, \
         tc.tile_pool(name="expsb", bufs=8) as expsb, \
         tc.tile_pool(name="otsb", bufs=4) as otsbp, \
         tc.tile_pool(name="recsb", bufs=4) as recp, \
         tc.tile_pool(name="rawsb", bufs=4) as rawp, \
         tc.tile_pool(name="outsb", bufs=2) as outsbp:

        # All DMAs ride the SP (sync) HWDGE ring: DMAs issued on nc.scalar
        # serialize on the ACT sequencer and delay the first exp by ~30us.
        # Per-c-block DMAs so projections stream behind the loads; order:
        # xkt, xqt chunk 0 (unblocks the first scores+exp), xvt (feeds
        # V-nat/AV), then the rest of xqt.
        xt = {}
        for name in ("xkt", "xqt", "xvt"):
            xt[name] = xinp.tile([128, NCC * T], bf16, tag=name, name=name)

        def xdma(eng, name, clo, chi):
            # one DMA covering c-blocks [clo, chi), all token cols
            nca = chi - clo
            eng.dma_start(
                xt[name][:, clo * T:chi * T]
                .rearrange("p (a t) -> p a t", a=nca),
                io[name][clo * 128:chi * 128, :]
                .rearrange("(a p) t -> p a t", p=128))

        # balance ~7 MB per HWDGE ring, both finishing v by ~2/3 through:
        # sync: q chunk 0 (gates first scores), k halves, second v half;
        # scalar (already carrying weights): first v half, rest of q
        nc.sync.dma_start(
            xt["xqt"][:].rearrange("p (a t) -> p a t", a=NCC)[:, :, 0:TCH],
            io["xqt"].rearrange("(a p) t -> p a t", p=128)[:, :, 0:TCH])
        for half in range(2):
            xdma(nc.sync, "xkt", half * NCC // 2, (half + 1) * NCC // 2)
        xdma(nc.scalar, "xvt", 0, NCC // 2)
        xdma(nc.sync, "xvt", NCC // 2, NCC)
        nc.scalar.dma_start(
            xt["xqt"][:].rearrange("p (a t) -> p a t", a=NCC)[:, :, TCH:T],
            io["xqt"].rearrange("(a p) t -> p a t", p=128)[:, :, TCH:T])

        def qk_proj(pool, tag, xname, wname, bname, XT, tci):
            xsb = xt[xname]
            for co in range(2):
                pj = pool.tile([128, TCH], f32, tag=tag, name="proj")
                for c in range(NCC):
                    nc.tensor.matmul(
                        pj[:],
                        lhsT=wsb[wname][c][:, co * 128:(co + 1) * 128],
                        rhs=xsb[:, c * T + tci * TCH:
                                c * T + (tci + 1) * TCH],
                        start=(c == 0), stop=(c == NCC - 1))
                nc.vector.tensor_scalar_add(
                    XT[co][:, tci * TCH:(tci + 1) * TCH],
                    pj[:], bias[bname][:, co:co + 1])

        with tc.tile_pool(name="pr", bufs=4, space="PSUM") as projps:
            for tci in range(NTCH):
                qk_proj(projps, "pr", "xkt", "wk", "bks", KT, tci)
            qk_proj(projps, "pr", "xqt", "wq", "bqs", QT, 0)

        sps = ctx.enter_context(tc.tile_pool(name="sps", bufs=2,
                                             space="PSUM"))
        otps = ctx.enter_context(tc.tile_pool(name="ot", bufs=4,
                                              space="PSUM"))

        def v_nat_tile(tt):
            # Vnat[t, ds] = sum_c xvT[c, t].T @ wv[c, ds]
            xv = xt["xvt"]
            bvb3 = bvb.rearrange("p (h d) -> p h d", h=HG)
            pv = otps.tile([128, DS], f32, tag="ot", name="vnat")
            for c in range(NCC):
                nc.tensor.matmul(
                    pv[:],
                    lhsT=xv[:, c * T + tt * 128:c * T + (tt + 1) * 128],
                    rhs=wsb["wv"][c],
                    start=(c == 0), stop=(c == NCC - 1))
            dst3 = VNB[:, tt * 512:(tt + 1) * 512].rearrange(
                "p (h c) -> p h c", h=HG)[:, :, 0:64]
            src3 = pv[:].rearrange("p (h d) -> p h d", h=HG)
            nc.vector.tensor_add(dst3, src3, bvb3)

        def attention(qc, interleave=None):
            # interleave(pr, kt): emitted inside the kt loop so the static
            # schedule folds extra PE work under the exp stream.
            # Returns the normalized [ds, q] bf16 halves for the output
            # projection (emitted later, interleaved under the next chunk).
            qcols = slice(qc * TCH, (qc + 1) * TCH)
            ot_sb = []
            for pr in range(2):
                otp = [otps.tile([128, TCH], f32, tag="ot", name="ot")
                       for _ in range(2)]
                for kt in range(NKT):
                    first, last = kt == 0, kt == NKT - 1
                    S = sps.tile([128, 2 * TCH], f32, tag="s", name="s")
                    for hh in range(2):
                        rows = slice(hh * 64, (hh + 1) * 64)
                        nc.tensor.matmul(
                            S[:, hh * TCH:(hh + 1) * TCH],
                            lhsT=KT[pr][rows, kt * 128:(kt + 1) * 128],
                            rhs=QT[pr][rows, qcols],
                            start=True, stop=True)
                    es = expsb.tile([128, 2 * TCH], bf16, tag="es",
                                    name="es")
                    nc.scalar.activation(es[:], S[:], EXP, scale=SCALE)
                    if interleave is not None:
                        interleave(pr, kt)
                    for hh in range(2):
                        h = pr * 2 + hh
                        nc.tensor.matmul(
                            otp[hh][:, :],
                            lhsT=VNB[:, kt * 512 + h * 128:
                                     kt * 512 + (h + 1) * 128],
                            rhs=es[:, hh * TCH:(hh + 1) * TCH],
                            start=first, stop=last)
                # copy accumulators to SBUF first (frees the psum
                # banks for the next pass), gathering BOTH heads' O rows
                # into one [128, 512] tile and both denominator blocks
                # into another: DVE reciprocal cost scales with free-dim
                # elements per lane only, so ONE full-partition reciprocal
                # (and one multiply) covers both heads at the cost of one
                # -- halving the ~8 cyc/elem iterative-divide budget.  It
                # runs with a full chunk of slack since the output
                # projection is deferred into the next chunk.
                rawo = rawp.tile([128, TCH], f32, tag="raw", name="rawo")
                den = recp.tile([128, TCH], f32, tag="rec", name="den")
                for hh in range(2):
                    rows = slice(hh * 64, (hh + 1) * 64)
                    nc.vector.tensor_copy(rawo[rows, :], otp[hh][0:64, :])
                    nc.vector.tensor_copy(den[rows, :], otp[hh][64:128, :])
                rec = recp.tile([128, TCH], f32, tag="rec", name="rec")
                nc.vector.reciprocal(rec[:], den[:])
                osb = otsbp.tile([128, TCH], bf16, tag="otsb",
                                 name="otsb")
                nc.vector.tensor_mul(osb[:], rawo[:], rec[:])
                ot_sb.append(osb)
            return ot_sb

        def make_outproj(qc, ot_sb):
            # one ct piece (2 matmuls + copy) per call; DMA per half
            qcols = slice(qc * TCH, (qc + 1) * TCH)
            nh = NCC // 2
            st = {"ob": None}

            def emit(step):
                ct = step[0]
                if ct >= NCC:
                    return
                half, idx = divmod(ct, nh)
                if idx == 0:
                    st["ob"] = outsbp.tile([128, nh * TCH], bf16,
                                           tag="ob", name="ob")
                pp = otps.tile([128, TCH], f32, tag="ot", name="prj")
                for dc in range(2):
                    nc.tensor.matmul(
                        pp[:],
                        lhsT=wot[dc][:, ct * 128:(ct + 1) * 128],
                        rhs=ot_sb[dc][:],
                        start=(dc == 0), stop=(dc == 1))
                nc.vector.tensor_copy(
                    st["ob"][:, idx * TCH:(idx + 1) * TCH], pp[:])
                if idx == nh - 1:
                    nc.sync.dma_start(
                        io["out_t"][half * nh * 128:(half + 1) * nh * 128,
                                    qcols]
                        .rearrange("(a p) t -> p a t", p=128),
                        st["ob"][:].rearrange("p (a t) -> p a t", a=nh))
                step[0] += 1
            return emit

        def spread_qproj(tci, co):
            # one c-block matmul per call; held psum accumulation across
            # calls (other-bank matmuls interleave freely)
            st = {"pj": None, "c": 0}

            def emit(_step=None):
                c = st["c"]
                if c >= NCC:
                    return
                if c == 0:
                    st["pj"] = otps.tile([128, TCH], f32, tag="ot",
                                         name="proj")
                nc.tensor.matmul(
                    st["pj"][:],
                    lhsT=wsb["wq"][c][:, co * 128:(co + 1) * 128],
                    rhs=xt["xqt"][:, c * T + tci * TCH:
                                  c * T + (tci + 1) * TCH],
                    start=(c == 0), stop=(c == NCC - 1))
                if c == NCC - 1:
                    nc.vector.tensor_scalar_add(
                        QT[co][:, tci * TCH:(tci + 1) * TCH],
                        st["pj"][:], bias["bqs"][:, co:co + 1])
                st["c"] += 1
            return emit

        def make_interleave(qc, out_prev):
            # fold under this chunk's exp stream: V-nat (qc0/pr0), the
            # PREVIOUS chunk's output projection (pr0, one ct piece per
            # kt: its normalize had a full chunk of slack, hiding the
            # expensive DVE reciprocal), the NEXT chunk's q projection
            # (pr1, bunched at kt 4/10 so its bias add queues ahead of
            # this chunk's reciprocals on the in-order DVE), and the wot
            # weight loads
            qn_co0 = spread_qproj(qc + 1, 0) if qc < NTCH - 1 else None
            qn_co1 = spread_qproj(qc + 1, 1) if qc < NTCH - 1 else None
            ostep = [0]

            def il(pr, kt):
                if qc == 0 and pr == 0:
                    v_nat_tile(kt)
                    if kt == 14:
                        nc.scalar.dma_start(wot[0][:], io["wot"][0:128, :])
                    if kt == 15:
                        nc.scalar.dma_start(wot[1][:], io["wot"][128:256, :])
                    return
                if pr == 0 and kt < NCC and out_prev is not None:
                    out_prev(ostep)
                if pr == 1 and kt == 4 and qn_co0 is not None:
                    for _ in range(NCC):
                        qn_co0()
                if pr == 1 and kt == 10 and qn_co1 is not None:
                    for _ in range(NCC):
                        qn_co1()
            return il

        out_prev = None
        for qc in range(NTCH):
            ot_sb = attention(qc, interleave=make_interleave(qc, out_prev))
            out_prev = make_outproj(qc, ot_sb)
        ostep = [0]
        for _ in range(NCC):
            out_prev(ostep)


def build_nc(reps=1):
    from contextlib import ExitStack

    import concourse.tile as tile
    from concourse import bacc, mybir

    f32 = mybir.dt.float32
    bf16 = mybir.dt.bfloat16
    nc = bacc.Bacc("TRN2", target_bir_lowering=False, debug=False,
                   num_devices=NCORES)
    io = {}
    for name in ("xqt", "xkt", "xvt"):
        io[name] = nc.dram_tensor(name, [C, T], bf16,
                                  kind="ExternalInput").ap()
    io["wqkv"] = nc.dram_tensor("wqkv", [C, 3 * DS], bf16,
                                kind="ExternalInput").ap()
    io["wot"] = nc.dram_tensor("wot", [DS, C], bf16, kind="ExternalInput").ap()
    io["biasp"] = nc.dram_tensor("biasp", [128, 4 + DS], f32,
                                 kind="ExternalInput").ap()
    io["out_t"] = nc.dram_tensor("out_t", [C, T], bf16,
                                 kind="ExternalOutput").ap()

    with tile.TileContext(nc) as tc:
        if reps == 1:
            with ExitStack() as ctx:
                _emit(ctx, tc, io)
        else:
            with tc.For_i(0, reps, 1):
                with ExitStack() as ctx:
                    _emit(ctx, tc, io)
    nc.compile()
    return nc


def get_nc():
    global _NC_CACHE
    if _NC_CACHE is None:
        _NC_CACHE = build_nc()
    return _NC_CACHE


def make_in_maps(q, k, v, Wq, bq, Wk, bk, Wv, bv, Wo, bo):
    import ml_dtypes

    bfdt = ml_dtypes.bfloat16
    q, k, v = (np.asarray(x, np.float32) for x in (q, k, v))
    Wq, Wk, Wv, Wo = (np.asarray(x, np.float32) for x in (Wq, Wk, Wv, Wo))
    bq, bk, bv, bo = (np.asarray(x, np.float32) for x in (bq, bk, bv, bo))
    # shared per-batch transposed activations (shared across 4 cores)
    xqt = [np.ascontiguousarray(q[b].T).astype(bfdt) for b in range(B)]
    xkt = [np.ascontiguousarray(k[b].T).astype(bfdt) for b in range(B)]
    xvt = [np.ascontiguousarray(v[b].T).astype(bfdt) for b in range(B)]
    in_maps = []
    for core in range(NCORES):
        b, g = divmod(core, GROUPS)
        sl = slice(g * DS, (g + 1) * DS)
        # packed weights: per row c, [wq | wk | wv] slices
        wqkv = np.concatenate(
            [Wq[sl, :].T, Wk[sl, :].T, Wv[sl, :].T], axis=1)
        # packed biases: [bqs(2) | bks(2) | bvb(DS)] per partition; the
        # bqs/bks columns hold the co-th 128-slice of the bias vector
        biasp = np.zeros((128, 4 + DS), np.float32)
        biasp[:, 0:2] = bq[sl].reshape(2, 128).T
        biasp[:, 2:4] = bk[sl].reshape(2, 128).T
        biasp[:, 4:] = np.broadcast_to(bv[sl], (128, DS))
        in_maps.append({
            "xqt": xqt[b],
            "xkt": xkt[b],
            "xvt": xvt[b],
            "wqkv": np.ascontiguousarray(wqkv).astype(bfdt),
            "wot": np.ascontiguousarray(Wo[:, sl].T).astype(bfdt),
            "biasp": biasp,
        })
    return in_maps


def combine(results, bo):
    out = np.zeros((B, T, C), np.float32)
    for core in range(NCORES):
        b, _ = divmod(core, GROUPS)
        out[b] += results[core]["out_t"].T.astype(np.float32)
    out += np.asarray(bo, np.float32)
    return out


def kernel(q, k, v, Wq, bq, Wk, bk, Wv, bv, Wo, bo):
    from concourse.bass_utils import run_bass_kernel_spmd

    nc = get_nc()
    in_maps = make_in_maps(q, k, v, Wq, bq, Wk, bk, Wv, bv, Wo, bo)
    res = run_bass_kernel_spmd(nc, in_maps, core_ids=list(range(NCORES)))
    return combine(res.results, bo)


# revision 57
# speedup vs baseline: 1.0914x; 1.0914x over previous
"""MultiHeadAttention Trainium2 kernel (8-core SPMD).

Problem: B=2, T=2048, C=1024, H=16 heads, D=64.
  out = softmax((q Wq^T + bq)(k Wk^T + bk)^T / sqrt(D)) (v Wv^T + bv) Wo^T + bo

Sharding: core c -> (batch b = c // 4, head-group g = c % 4).  Each core
computes 4 heads (a 256-wide slice of the projection space) of one batch
element, including its partial contribution to the row-sharded output
projection.  The host sums the 4 partial outputs per batch and adds bo.

Design (measured ~292 us vs the 507 us v1 baseline):
  - Activations are pre-transposed AND pre-cast to bf16 on the HOST:
    xqt/xkt/xvt = x[b].T as [C, T] bf16.  No PE transposes, no PSUM->SBUF
    staging copies, and half the input DMA bytes.
  - Q/K projections: QT/KT[co, t] = W^T.T @ xT directly (bf16 matmul,
    fp32 psum), stored f32r with bias folded via DVE tensor_scalar_add.
  - V projected in NATURAL orientation (tokens on partitions):
    Vnat[t, ds] = xvT_tile.T @ Wv^T — feeds AV without any transpose.
    VN layout per key-tile: [4 heads x (64 V cols + 64 ones cols)]; the
    ones columns make each AV matmul also emit the softmax denominator.
  - S^T[k, q] per head pair: two row-packed matmuls (partitions 0:64 /
    64:128 -> concurrent PE row-groups) into one [128, 1024] psum pair;
    ONE exp activation covers both heads (bf16 out, scale=1/8 folded).
  - Output projection accumulates W_o slices over the 256 head dims,
    written bf16; bo is added on the host during the combine.
"""

import numpy as np

B, T, C, H, D = 2, 2048, 1024, 16, 64
NCORES = 8
GROUPS = 4              # head-groups == cores per batch element
HG = H // GROUPS        # heads per core
DS = HG * D             # per-core projection slice width (256)
TCH = 512               # token chunk (psum bank = 512 fp32)
NTCH = T // TCH         # 4
NCC = C // 128          # 8 contraction chunks
NKT = T // 128          # 16 key tiles
SCALE = float(D) ** -0.5

_NC_CACHE = None


def _emit(ctx, tc, io):
    from concourse import mybir

    nc = tc.nc
    f32 = mybir.dt.float32
    f32r = mybir.dt.float32r
    bf16 = mybir.dt.bfloat16
    EXP = mybir.ActivationFunctionType.Exp

    persist = ctx.enter_context(tc.tile_pool(name="persist", bufs=1))

    def ptile(tag, shape, dt=f32):
        return persist.tile(shape, dt, tag=tag, name=tag)

    # --- persistent SBUF tensors ---------------------------------------
    # wqkv [C, 3*DS] = wq|wk|wv packed -> ONE weight DMA; biasp [128, 260]
    # = bqs|bks|bvb packed -> one more.  Both ride the scalar HWDGE ring
    # (ACT is idle until the first exp anyway) so the sync ring starts on
    # xkt immediately.
    wqkv = ptile("wqkv", [128, NCC * 3 * DS], bf16)
    nc.scalar.dma_start(
        wqkv[:].rearrange("p (a s) -> p a s", a=NCC),
        io["wqkv"].rearrange("(a p) s -> p a s", p=128))
    wsb = {}
    for i, name in enumerate(("wq", "wk", "wv")):
        wsb[name] = [
            wqkv[:, (c * 3 + i) * DS:(c * 3 + i + 1) * DS]
            for c in range(NCC)
        ]
    # wot DMAs are emitted late (just before attention) so they queue
    # behind the x loads instead of delaying them
    wot = [ptile(f"wot{dc}", [128, C], bf16) for dc in range(2)]

    biasp = ptile("biasp", [128, 4 + DS])
    nc.scalar.dma_start(biasp[:], io["biasp"][:, :])
    bias = {"bqs": biasp[:, 0:2], "bks": biasp[:, 2:4]}
    bvb = biasp[:, 4:4 + DS]

    QT = [ptile(f"qt{i}", [128, T], f32r) for i in range(2)]
    KT = [ptile(f"kt{i}", [128, T], f32r) for i in range(2)]
    # V natural, all 16 key tiles in one buffer; per key tile the layout is
    # [4 heads x (64 V cols + 64 ones cols)].  Ones prefilled via memset;
    # V columns overwritten when the V projection lands.
    VNB = ptile("vnb", [128, NKT * 512], bf16)
    nc.gpsimd.memset(VNB[:], 1.0)

    # --- stage A + B, software-pipelined -------------------------------
    # DMA order: weights (scalar), xkt (sync), xqt (scalar), xvt (sync):
    # scores for q-chunk 0 only need K fully projected + Q chunk 0, so the
    # exp stream starts as soon as K + Q0 land; V arrives third and feeds
    # the AV matmuls (the 6-deep es pool bridges the exp->AV lag).
    # PSUM: projections 4 banks (pool closed before attention), then
    # scores 2x[128,1024] (4 banks) + 4 AV-accumulator banks (the output
    # projection borrows freed accumulator slots).
    with tc.tile_pool(name="xin", bufs=1) as xinp, \
         tc.tile_pool(name="expsb", bufs=8) as expsb, \
         tc.tile_pool(name="otsb", bufs=4) as otsbp, \
         tc.tile_pool(name="recsb", bufs=4) as recp, \
         tc.tile_pool(name="rawsb", bufs=4) as rawp, \
         tc.tile_pool(name="outsb", bufs=2) as outsbp:

        # All DMAs ride the SP (sync) HWDGE ring: DMAs issued on nc.scalar
        # serialize on the ACT sequencer and delay the first exp by ~30us.
        # Per-c-block DMAs so projections stream behind the loads; order:
        # xkt, xqt chunk 0 (unblocks the first scores+exp), xvt (feeds
        # V-nat/AV), then the rest of xqt.
        xt = {}
        for name in ("xkt", "xqt", "xvt"):
            xt[name] = xinp.tile([128, NCC * T], bf16, tag=name, name=name)

        def xdma(eng, name, clo, chi):
            # one DMA covering c-blocks [clo, chi), all token cols
            nca = chi - clo
            eng.dma_start(
                xt[name][:, clo * T:chi * T]
                .rearrange("p (a t) -> p a t", a=nca),
                io[name][clo * 128:chi * 128, :]
                .rearrange("(a p) t -> p a t", p=128))

        # balance ~7 MB per HWDGE ring, both finishing v by ~2/3 through:
        # sync: q chunk 0 (gates first scores), k halves, second v half;
        # scalar (already carrying weights): first v half, rest of q
        nc.sync.dma_start(
            xt["xqt"][:].rearrange("p (a t) -> p a t", a=NCC)[:, :, 0:TCH],
            io["xqt"].rearrange("(a p) t -> p a t", p=128)[:, :, 0:TCH])
        # xkt split by token columns: chunk 0 (1 MB) lands first so the
        # first K projection chunk -- and with it the first scores+exp --
        # clears ~10us earlier; the remaining columns follow
        nc.sync.dma_start(
            xt["xkt"][:].rearrange("p (a t) -> p a t", a=NCC)[:, :, 0:TCH],
            io["xkt"].rearrange("(a p) t -> p a t", p=128)[:, :, 0:TCH])
        nc.sync.dma_start(
            xt["xkt"][:].rearrange("p (a t) -> p a t", a=NCC)[:, :, TCH:T],
            io["xkt"].rearrange("(a p) t -> p a t", p=128)[:, :, TCH:T])
        xdma(nc.scalar, "xvt", 0, NCC // 2)
        xdma(nc.sync, "xvt", NCC // 2, NCC)
        nc.scalar.dma_start(
            xt["xqt"][:].rearrange("p (a t) -> p a t", a=NCC)[:, :, TCH:T],
            io["xqt"].rearrange("(a p) t -> p a t", p=128)[:, :, TCH:T])

        def qk_proj(pool, tag, xname, wname, bname, XT, tci):
            xsb = xt[xname]
            for co in range(2):
                pj = pool.tile([128, TCH], f32, tag=tag, name="proj")
                for c in range(NCC):
                    nc.tensor.matmul(
                        pj[:],
                        lhsT=wsb[wname][c][:, co * 128:(co + 1) * 128],
                        rhs=xsb[:, c * T + tci * TCH:
                                c * T + (tci + 1) * TCH],
                        start=(c == 0), stop=(c == NCC - 1))
                nc.vector.tensor_scalar_add(
                    XT[co][:, tci * TCH:(tci + 1) * TCH],
                    pj[:], bias[bname][:, co:co + 1])

        with tc.tile_pool(name="pr", bufs=4, space="PSUM") as projps:
            qk_proj(projps, "pr", "xkt", "wk", "bks", KT, 0)
            qk_proj(projps, "pr", "xqt", "wq", "bqs", QT, 0)
            for tci in range(1, NTCH):
                qk_proj(projps, "pr", "xkt", "wk", "bks", KT, tci)

        sps = ctx.enter_context(tc.tile_pool(name="sps", bufs=2,
                                             space="PSUM"))
        otps = ctx.enter_context(tc.tile_pool(name="ot", bufs=4,
                                              space="PSUM"))

        def v_nat_tile(tt):
            # Vnat[t, ds] = sum_c xvT[c, t].T @ wv[c, ds]
            xv = xt["xvt"]
            bvb3 = bvb.rearrange("p (h d) -> p h d", h=HG)
            pv = otps.tile([128, DS], f32, tag="ot", name="vnat")
            for c in range(NCC):
                nc.tensor.matmul(
                    pv[:],
                    lhsT=xv[:, c * T + tt * 128:c * T + (tt + 1) * 128],
                    rhs=wsb["wv"][c],
                    start=(c == 0), stop=(c == NCC - 1))
            dst3 = VNB[:, tt * 512:(tt + 1) * 512].rearrange(
                "p (h c) -> p h c", h=HG)[:, :, 0:64]
            src3 = pv[:].rearrange("p (h d) -> p h d", h=HG)
            nc.vector.tensor_add(dst3, src3, bvb3)

        def attention(qc, interleave=None):
            # interleave(pr, kt): emitted inside the kt loop so the static
            # schedule folds extra PE work under the exp stream.
            # Returns the normalized [ds, q] bf16 halves for the output
            # projection (emitted later, interleaved under the next chunk).
            qcols = slice(qc * TCH, (qc + 1) * TCH)
            ot_sb = []
            for pr in range(2):
                otp = [otps.tile([128, TCH], f32, tag="ot", name="ot")
                       for _ in range(2)]
                for kt in range(NKT):
                    first, last = kt == 0, kt == NKT - 1
                    S = sps.tile([128, 2 * TCH], f32, tag="s", name="s")
                    for hh in range(2):
                        rows = slice(hh * 64, (hh + 1) * 64)
                        nc.tensor.matmul(
                            S[:, hh * TCH:(hh + 1) * TCH],
                            lhsT=KT[pr][rows, kt * 128:(kt + 1) * 128],
                            rhs=QT[pr][rows, qcols],
                            start=True, stop=True)
                    es = expsb.tile([128, 2 * TCH], bf16, tag="es",
                                    name="es")
                    nc.scalar.activation(es[:], S[:], EXP, scale=SCALE)
                    if interleave is not None:
                        interleave(pr, kt)
                    for hh in range(2):
                        h = pr * 2 + hh
                        nc.tensor.matmul(
                            otp[hh][:, :],
                            lhsT=VNB[:, kt * 512 + h * 128:
                                     kt * 512 + (h + 1) * 128],
                            rhs=es[:, hh * TCH:(hh + 1) * TCH],
                            start=first, stop=last)
                # copy accumulators to SBUF first (frees the psum
                # banks for the next pass), gathering BOTH heads' O rows
                # into one [128, 512] tile and both denominator blocks
                # into another: DVE reciprocal cost scales with free-dim
                # elements per lane only, so ONE full-partition reciprocal
                # (and one multiply) covers both heads at the cost of one
                # -- halving the ~8 cyc/elem iterative-divide budget.  It
                # runs with a full chunk of slack since the output
                # projection is deferred into the next chunk.
                rawo = rawp.tile([128, TCH], f32, tag="raw", name="rawo")
                den = recp.tile([128, TCH], f32, tag="rec", name="den")
                for hh in range(2):
                    rows = slice(hh * 64, (hh + 1) * 64)
                    nc.vector.tensor_copy(rawo[rows, :], otp[hh][0:64, :])
                    nc.vector.tensor_copy(den[rows, :], otp[hh][64:128, :])
                rec = recp.tile([128, TCH], f32, tag="rec", name="rec")
                nc.vector.reciprocal(rec[:], den[:])
                osb = otsbp.tile([128, TCH], bf16, tag="otsb",
                                 name="otsb")
                nc.vector.tensor_mul(osb[:], rawo[:], rec[:])
                ot_sb.append(osb)
            return ot_sb

        def make_outproj(qc, ot_sb):
            # one ct piece (2 matmuls + copy) per call; DMA per half
            qcols = slice(qc * TCH, (qc + 1) * TCH)
            nh = NCC // 2
            st = {"ob": None}

            def emit(step):
                ct = step[0]
                if ct >= NCC:
                    return
                half, idx = divmod(ct, nh)
                if idx == 0:
                    st["ob"] = outsbp.tile([128, nh * TCH], bf16,
                                           tag="ob", name="ob")
                pp = otps.tile([128, TCH], f32, tag="ot", name="prj")
                for dc in range(2):
                    nc.tensor.matmul(
                        pp[:],
                        lhsT=wot[dc][:, ct * 128:(ct + 1) * 128],
                        rhs=ot_sb[dc][:],
                        start=(dc == 0), stop=(dc == 1))
                nc.vector.tensor_copy(
                    st["ob"][:, idx * TCH:(idx + 1) * TCH], pp[:])
                if idx == nh - 1:
                    nc.sync.dma_start(
                        io["out_t"][half * nh * 128:(half + 1) * nh * 128,
                                    qcols]
                        .rearrange("(a p) t -> p a t", p=128),
                        st["ob"][:].rearrange("p (a t) -> p a t", a=nh))
                step[0] += 1
            return emit

        def spread_qproj(tci, co):
            # one c-block matmul per call; held psum accumulation across
            # calls (other-bank matmuls interleave freely)
            st = {"pj": None, "c": 0}

            def emit(_step=None):
                c = st["c"]
                if c >= NCC:
                    return
                if c == 0:
                    st["pj"] = otps.tile([128, TCH], f32, tag="ot",
                                         name="proj")
                nc.tensor.matmul(
                    st["pj"][:],
                    lhsT=wsb["wq"][c][:, co * 128:(co + 1) * 128],
                    rhs=xt["xqt"][:, c * T + tci * TCH:
                                  c * T + (tci + 1) * TCH],
                    start=(c == 0), stop=(c == NCC - 1))
                if c == NCC - 1:
                    nc.vector.tensor_scalar_add(
                        QT[co][:, tci * TCH:(tci + 1) * TCH],
                        st["pj"][:], bias["bqs"][:, co:co + 1])
                st["c"] += 1
            return emit

        def make_interleave(qc, out_prev):
            # fold under this chunk's exp stream: V-nat (qc0/pr0), the
            # PREVIOUS chunk's output projection (pr0, one ct piece per
            # kt: its normalize had a full chunk of slack, hiding the
            # expensive DVE reciprocal), the NEXT chunk's q projection
            # (pr1, bunched at kt 4/10 so its bias add queues ahead of
            # this chunk's reciprocals on the in-order DVE), and the wot
            # weight loads
            qn_co0 = spread_qproj(qc + 1, 0) if qc < NTCH - 1 else None
            qn_co1 = spread_qproj(qc + 1, 1) if qc < NTCH - 1 else None
            ostep = [0]

            def il(pr, kt):
                if qc == 0 and pr == 0:
                    v_nat_tile(kt)
                    if kt == 14:
                        nc.scalar.dma_start(wot[0][:], io["wot"][0:128, :])
                    if kt == 15:
                        nc.scalar.dma_start(wot[1][:], io["wot"][128:256, :])
                    return
                if pr == 0 and kt < NCC and out_prev is not None:
                    out_prev(ostep)
                if pr == 1 and kt == 4 and qn_co0 is not None:
                    for _ in range(NCC):
                        qn_co0()
                if pr == 1 and kt == 10 and qn_co1 is not None:
                    for _ in range(NCC):
                        qn_co1()
            return il

        out_prev = None
        for qc in range(NTCH):
            ot_sb = attention(qc, interleave=make_interleave(qc, out_prev))
            out_prev = make_outproj(qc, ot_sb)
        ostep = [0]
        for _ in range(NCC):
            out_prev(ostep)


def build_nc(reps=1):
    from contextlib import ExitStack

    import concourse.tile as tile
    from concourse import bacc, mybir

    f32 = mybir.dt.float32
    bf16 = mybir.dt.bfloat16
    nc = bacc.Bacc("TRN2", target_bir_lowering=False, debug=False,
                   num_devices=NCORES)
    io = {}
    for name in ("xqt", "xkt", "xvt"):
        io[name] = nc.dram_tensor(name, [C, T], bf16,
                                  kind="ExternalInput").ap()
    io["wqkv"] = nc.dram_tensor("wqkv", [C, 3 * DS], bf16,
                                kind="ExternalInput").ap()
    io["wot"] = nc.dram_tensor("wot", [DS, C], bf16, kind="ExternalInput").ap()
    io["biasp"] = nc.dram_tensor("biasp", [128, 4 + DS], f32,
                                 kind="ExternalInput").ap()
    io["out_t"] = nc.dram_tensor("out_t", [C, T], bf16,
                                 kind="ExternalOutput").ap()

    with tile.TileContext(nc) as tc:
        if reps == 1:
            with ExitStack() as ctx:
                _emit(ctx, tc, io)
        else:
            with tc.For_i(0, reps, 1):
                with ExitStack() as ctx:
                    _emit(ctx, tc, io)
    nc.compile()
    return nc


def get_nc():
    global _NC_CACHE
    if _NC_CACHE is None:
        _NC_CACHE = build_nc()
    return _NC_CACHE


def make_in_maps(q, k, v, Wq, bq, Wk, bk, Wv, bv, Wo, bo):
    import ml_dtypes

    bfdt = ml_dtypes.bfloat16
    q, k, v = (np.asarray(x, np.float32) for x in (q, k, v))
    Wq, Wk, Wv, Wo = (np.asarray(x, np.float32) for x in (Wq, Wk, Wv, Wo))
    bq, bk, bv, bo = (np.asarray(x, np.float32) for x in (bq, bk, bv, bo))
    # shared per-batch transposed activations (shared across 4 cores)
    xqt = [np.ascontiguousarray(q[b].T).astype(bfdt) for b in range(B)]
    xkt = [np.ascontiguousarray(k[b].T).astype(bfdt) for b in range(B)]
    xvt = [np.ascontiguousarray(v[b].T).astype(bfdt) for b in range(B)]
    in_maps = []
    for core in range(NCORES):
        b, g = divmod(core, GROUPS)
        sl = slice(g * DS, (g + 1) * DS)
        # packed weights: per row c, [wq | wk | wv] slices
        wqkv = np.concatenate(
            [Wq[sl, :].T, Wk[sl, :].T, Wv[sl, :].T], axis=1)
        # packed biases: [bqs(2) | bks(2) | bvb(DS)] per partition; the
        # bqs/bks columns hold the co-th 128-slice of the bias vector
        biasp = np.zeros((128, 4 + DS), np.float32)
        biasp[:, 0:2] = bq[sl].reshape(2, 128).T
        biasp[:, 2:4] = bk[sl].reshape(2, 128).T
        biasp[:, 4:] = np.broadcast_to(bv[sl], (128, DS))
        in_maps.append({
            "xqt": xqt[b],
            "xkt": xkt[b],
            "xvt": xvt[b],
            "wqkv": np.ascontiguousarray(wqkv).astype(bfdt),
            "wot": np.ascontiguousarray(Wo[:, sl].T).astype(bfdt),
            "biasp": biasp,
        })
    return in_maps


def combine(results, bo):
    out = np.zeros((B, T, C), np.float32)
    for core in range(NCORES):
        b, _ = divmod(core, GROUPS)
        out[b] += results[core]["out_t"].T.astype(np.float32)
    out += np.asarray(bo, np.float32)
    return out


def kernel(q, k, v, Wq, bq, Wk, bk, Wv, bv, Wo, bo):
    from concourse.bass_utils import run_bass_kernel_spmd

    nc = get_nc()
    in_maps = make_in_maps(q, k, v, Wq, bq, Wk, bk, Wv, bv, Wo, bo)
    res = run_bass_kernel_spmd(nc, in_maps, core_ids=list(range(NCORES)))
    return combine(res.results, bo)
